# revision 26
# baseline (speedup 1.0000x reference)
"""GPT decoder (B=8,T=256,D=512,H=8,L=6,DFF=2048,V=50257) on 8 TRN2 NeuronCores.

Strategy (v3):
- Data-parallel over batch (core c owns batch c); vocab projection is
  tensor-parallel over vocab (per-core shard 6284, padded to 6400).
- LN gains/biases folded into weights host-side (exact algebra); on-chip LN
  only computes (x-mean)*rstd, with rstd = exp(-0.5*ln(var+eps)) so the
  whole kernel uses a single ACT table set (natural_log_exp_and_others).
- Residual stream lives in PSUM (xp0/xp1, one bank per token tile):
  Wo/W2/bias adds are matmul accumulations; LN reads PSUM directly.
- Tiles are split per token-tile / head-pair / dff-chunk so Tile's
  tile-granular dependency tracking doesn't serialize independent work
  (LN of tile 0 overlaps the previous FFN's tile-1 matmuls, etc).
- Causal skip; causal mask preloaded into PSUM via identity-matmul.
- fp16 activations/weights; fp16 logits (halved output DMA).
- Vocab projection computed transposed (logits^T) so bout is a
  per-partition bias on the drains (split ACT/DVE); Wout tile stationary
  across 4 token-slot matmuls; contiguous fp16 output tiles.
- AllGather split in 2 token-chunks; vocab runs chunk-0 token slots first
  so chunk 1 gathers behind compute.
"""
import math
import os

import numpy as np

import concourse.bass as bass
import concourse.tile as tile
from concourse import bacc, mybir
from concourse import bass_utils
from concourse.masks import make_identity

F32 = mybir.dt.float32
F16 = mybir.dt.float16
I32 = mybir.dt.int32
AF = mybir.ActivationFunctionType

D = 512
T = 256
H = 8
DK = 64
L = 6
DFF = 2048
V = 50257
B = 8
NCORES = 8
P = 128

TT = 2              # token tiles (T / P)
KB = D // P         # 4 contraction chunks over D
FB = DFF // P       # 16 chunks over DFF
VS = 6284           # per-core vocab shard (8 * 6284 = 50272 >= 50257)
VC = 50             # padded vocab 128-chunks per core (6400)
TS = 4              # 512-token slots in vocab matmul
VEARLY = 6          # vocab chunks that run ts{0,1} first (hide AG chunk 1)
NEGMASK = -60000.0  # fp16-safe -inf surrogate

_CACHE: dict = {}


def _steer_act_tables():
    """Make the ACT-table-load pass resolve every activation to the one
    set that contains all funcs this kernel uses (exp, ln, relu, identity,
    copy). Without this, Ln and Exp resolve to two different sets and the
    program ping-pongs table loads (~1.3us each) every layernorm."""
    from concourse import hw_specs
    if getattr(hw_specs.get_activation_tables, "_steered", False):
        return
    orig = hw_specs.get_activation_tables
    KEEP = "natural_log_exp_and_others"

    @__import__("functools").cache
    def patched(arch):
        tabs = dict(orig(arch))
        return {name: (funcs if name == KEEP else set())
                for name, funcs in tabs.items()}

    patched._steered = True
    hw_specs.get_activation_tables = patched
    bacc.get_activation_tables = patched


def _build_program():
    _steer_act_tables()
    nc = bacc.Bacc("TRN2", target_bir_lowering=False, debug=False,
                   num_devices=NCORES)

    # ---- I/O ----
    idx_h = nc.dram_tensor("idx", [T, 1], I32, kind="ExternalInput")
    emb_h = nc.dram_tensor("emb", [V, D], F16, kind="ExternalInput")
    posenc_h = nc.dram_tensor("posenc", [T, D], F32, kind="ExternalInput")
    mask_h = nc.dram_tensor("maskadd", [TT, P, T], F16, kind="ExternalInput")
    wqkv_h = nc.dram_tensor("wqkv", [L, 3, P, KB, D], F16, kind="ExternalInput")
    wo_h = nc.dram_tensor("wo", [L, P, KB, D], F16, kind="ExternalInput")
    w1_h = nc.dram_tensor("w1", [L, P, KB, DFF], F16, kind="ExternalInput")
    w2_h = nc.dram_tensor("w2", [L, P, FB, D], F16, kind="ExternalInput")
    # packed per-layer small consts: cq[4], ck[4], b1'[16]  (per partition)
    smallw_h = nc.dram_tensor("smallw", [L, P, 2 * KB + FB], F32,
                              kind="ExternalInput")
    # per-layer single-row consts: cv[512], b2[512]
    rows_h = nc.dram_tensor("rows", [L, 2, D], F16, kind="ExternalInput")
    wout_h = nc.dram_tensor("wout", [VC, P, KB * P], F16, kind="ExternalInput")
    bout_h = nc.dram_tensor("bout", [P, VC], F32, kind="ExternalInput")
    logits_h = nc.dram_tensor("logits", [VC, TS, P, 512], F16,
                              kind="ExternalOutput")

    scale = 1.0 / math.sqrt(D)

    with tile.TileContext(nc) as tc:
        from contextlib import ExitStack
        with ExitStack() as ctx:
            consts = ctx.enter_context(tc.tile_pool(name="consts", bufs=1))
            acts = ctx.enter_context(tc.tile_pool(name="acts", bufs=1))
            scr = ctx.enter_context(tc.tile_pool(name="scr", bufs=4))
            scr2 = ctx.enter_context(tc.tile_pool(name="scr2", bufs=4))
            dram = ctx.enter_context(tc.tile_pool(name="dram", bufs=1, space="DRAM"))

            # ---- constants ----
            identB = consts.tile([P, P], F16)
            make_identity(nc, identB)
            identF = consts.tile([P, P], F32)
            make_identity(nc, identF)
            ones1 = consts.tile([1, P], F16)
            nc.vector.memset(ones1, 1.0)
            eps_sb = consts.tile([P, 1], F32)
            nc.vector.memset(eps_sb, 1e-5)
            posenc_sb = consts.tile([P, TT, D], F32)
            nc.sync.dma_start(out=posenc_sb,
                              in_=posenc_h.ap().rearrange("(t p) d -> p t d", p=P))
            mask_sb = consts.tile([P, TT, T], F16)
            nc.sync.dma_start(out=mask_sb,
                              in_=mask_h.ap().rearrange("t p s -> p t s"))
            bout_sb = consts.tile([P, VC], F32)
            nc.sync.dma_start(out=bout_sb, in_=bout_h.ap())

            # ---- persistent activations (split to avoid false deps) ----
            xns = [acts.tile([P, D], F16, name=f"xn{t}") for t in range(TT)]
            xnT = [acts.tile([P, T], F16, name=f"xnT{k}") for k in range(KB)]
            qtp = [acts.tile([P, T], F16, name=f"qtp{k}") for k in range(KB)]
            ktp = [acts.tile([P, T], F16, name=f"ktp{k}") for k in range(KB)]
            vvs = [acts.tile([P, D], F16, name=f"vv{t}") for t in range(TT)]
            otp = [acts.tile([P, T], F16, name=f"otp{k}") for k in range(KB)]
            htf = [acts.tile([P, T], F16, name=f"htf{f}") for f in range(FB)]
            xsb = [acts.tile([P, D], F32, name=f"xsb{t}") for t in range(TT)]

            # ---- PSUM pools (layer phase; closed before the vocab phase) ----
            lps = ExitStack()
            xp_pool = lps.enter_context(tc.tile_pool(name="xp", bufs=1, space="PSUM"))
            pA = lps.enter_context(tc.tile_pool(name="pA", bufs=3, space="PSUM"))
            pB = lps.enter_context(tc.tile_pool(name="pB", bufs=3, space="PSUM"))
            xps = [xp_pool.tile([P, D], F32, name=f"xp{t}") for t in range(TT)]

            # ---- embedding + positional ----
            idx_sb = acts.tile([P, TT], I32)
            nc.sync.dma_start(out=idx_sb,
                              in_=idx_h.ap().rearrange("(t p) one -> p (t one)", p=P))
            for t in range(TT):
                emb_g = scr.tile([P, D], F16, name="emb_g")
                nc.gpsimd.indirect_dma_start(
                    out=emb_g[:], out_offset=None,
                    in_=emb_h.ap(),
                    in_offset=bass.IndirectOffsetOnAxis(ap=idx_sb[:, t:t + 1], axis=0),
                )
                xe = scr.tile([P, D], F32, name="xe")
                nc.vector.tensor_add(out=xe, in0=emb_g, in1=posenc_sb[:, t])
                # x[t] = xe  (fp32 identity matmul; sets has_written)
                nc.tensor.matmul(xps[t][:], identF[:], xe[:], start=True, stop=True)

            def layernorm(t, src=None):
                """(x[t]-mean)*rstd -> xns[t] (fp16). rstd via ln+exp.
                t==0 normalizes on DVE; t==1 on ACT (parallel engines)."""
                xs = xps[t] if src is None else src[t]
                stats = scr.tile([P, 6], F32, name="ln_stats")
                nc.vector.bn_stats(out=stats, in_=xs[:])
                mv = scr.tile([P, 2], F32, name="ln_mv")
                nc.vector.bn_aggr(out=mv, in_=stats)
                lv = scr.tile([P, 1], F32, name="ln_lv")
                nc.scalar.activation(out=lv, in_=mv[:, 1:2], func=AF.Ln,
                                     bias=eps_sb, scale=1.0)
                rstd = scr.tile([P, 1], F32, name="ln_rstd")
                nc.scalar.activation(out=rstd, in_=lv, func=AF.Exp,
                                     scale=-0.5)
                if t == 0:
                    nc.vector.tensor_scalar(out=xns[t][:], in0=xs[:],
                                            scalar1=mv[:, 0:1], scalar2=rstd,
                                            op0=mybir.AluOpType.subtract,
                                            op1=mybir.AluOpType.mult)
                else:
                    nb = scr.tile([P, 1], F32, name="ln_nb")
                    nc.vector.tensor_scalar(out=nb, in0=mv[:, 0:1],
                                            scalar1=rstd, scalar2=-1.0,
                                            op0=mybir.AluOpType.mult,
                                            op1=mybir.AluOpType.mult)
                    nc.scalar.activation(out=xns[t][:], in_=xs[:],
                                         func=AF.Identity, bias=nb, scale=rstd)

            def transpose_tile(t):
                """xns[t] -> xnT[kb][:, t*128:(t+1)*128]"""
                for kb in range(KB):
                    tp = pA.tile([P, P], F16, name="psA")
                    nc.tensor.transpose(out=tp[:],
                                        in_=xns[t][:, kb * P:(kb + 1) * P],
                                        identity=identB[:])
                    if kb % 2 == 0:
                        nc.vector.tensor_copy(out=xnT[kb][:, t * P:(t + 1) * P],
                                              in_=tp[:])
                    else:
                        nc.scalar.copy(out=xnT[kb][:, t * P:(t + 1) * P],
                                       in_=tp[:])

            # ================= decoder layers =================
            LEFF = 0 if os.environ.get("KERNEL_BISECT") == "nolayers" else L
            with tc.tile_pool(name="wpool", bufs=2) as wp:
                for l in range(LEFF):
                    wqkv_t = wp.tile([P, 3, KB, D], F16, name="wqkv_t")
                    for m in range(3):
                        nc.sync.dma_start(out=wqkv_t[:, m], in_=wqkv_h.ap()[l, m])
                    wo_t = wp.tile([P, KB, D], F16, name="wo_t")
                    nc.sync.dma_start(out=wo_t, in_=wo_h.ap()[l])
                    w1_t = wp.tile([P, KB, DFF], F16, name="w1_t")
                    nc.sync.dma_start(out=w1_t, in_=w1_h.ap()[l])
                    w2_t = wp.tile([P, FB, D], F16, name="w2_t")
                    nc.sync.dma_start(out=w2_t, in_=w2_h.ap()[l])
                    sw = wp.tile([P, 2 * KB + FB], F32, name="sw")
                    nc.sync.dma_start(out=sw, in_=smallw_h.ap()[l])
                    rows_sb = wp.tile([1, 2, D], F16, name="rows_sb")
                    nc.sync.dma_start(out=rows_sb, in_=rows_h.ap()[l])

                    # ---- LN1; V(t0) matmuls bridge the t1 LN chain ----
                    layernorm(0)
                    transpose_tile(0)
                    layernorm(1)
                    vps0 = pB.tile([P, D], F32, name="psB")
                    for kb in range(KB):
                        nc.tensor.matmul(vps0[:], xnT[kb][:, 0:P],
                                         wqkv_t[:, 2, kb],
                                         start=(kb == 0), stop=False)
                    nc.tensor.matmul(vps0[:], ones1[:], rows_sb[:, 0],
                                     start=False, stop=True)
                    nc.vector.tensor_copy(out=vvs[0][:], in_=vps0[:])
                    transpose_tile(1)

                    # ---- Q^T, K^T with folded-LN bias on the drain ----
                    for m, dst in ((0, qtp), (1, ktp)):
                        for pair in range(KB):
                            ps = pA.tile([P, T], F32, name="psA")
                            for kb in range(KB):
                                nc.tensor.matmul(
                                    ps[:],
                                    wqkv_t[:, m, kb, pair * P:(pair + 1) * P],
                                    xnT[kb][:],
                                    start=(kb == 0), stop=(kb == KB - 1))
                            bias_ap = sw[:, m * KB + pair:m * KB + pair + 1]
                            if pair % 2 == 0:
                                nc.scalar.activation(out=dst[pair][:], in_=ps[:],
                                                     func=AF.Identity,
                                                     bias=bias_ap, scale=1.0)
                            else:
                                nc.vector.tensor_scalar_add(out=dst[pair][:],
                                                            in0=ps[:],
                                                            scalar1=bias_ap)
                    # ---- V(t1) ----
                    vps1 = pB.tile([P, D], F32, name="psB")
                    for kb in range(KB):
                        nc.tensor.matmul(vps1[:], xnT[kb][:, P:T],
                                         wqkv_t[:, 2, kb],
                                         start=(kb == 0), stop=False)
                    nc.tensor.matmul(vps1[:], ones1[:], rows_sb[:, 0],
                                     start=False, stop=True)
                    nc.scalar.copy(out=vvs[1][:], in_=vps1[:])

                    # ---- attention ----
                    for pair in range(KB):
                        at_pair = []
                        for sub in range(2):
                            off = sub * DK
                            s_ps = pB.tile([P, TT, T], F32, name="psB")
                            nc.tensor.matmul(s_ps[:, 0, 0:P], identB[:],
                                             mask_sb[:, 0, 0:P],
                                             start=True, stop=False)
                            nc.tensor.matmul(
                                s_ps[:, 0, 0:P],
                                qtp[pair][off:off + DK, 0:P],
                                ktp[pair][off:off + DK, 0:P],
                                start=False, stop=True)
                            nc.tensor.matmul(s_ps[:, 1], identB[:],
                                             mask_sb[:, 1],
                                             start=True, stop=False)
                            nc.tensor.matmul(
                                s_ps[:, 1],
                                qtp[pair][off:off + DK, P:T],
                                ktp[pair][off:off + DK, :],
                                start=False, stop=True)

                            at = scr2.tile([P, TT, T], F16, name="at")
                            for tq in range(TT):
                                ncols = P if tq == 0 else T
                                pexp = scr.tile([P, T], F32, name="pexp")
                                den = scr.tile([P, 1], F32, name="den")
                                nc.scalar.activation(
                                    out=pexp[:, :ncols], in_=s_ps[:, tq, :ncols],
                                    func=AF.Exp, scale=scale, accum_out=den)
                                rden = scr.tile([P, 1], F32, name="rden")
                                nc.vector.reciprocal(out=rden, in_=den)
                                a_bf = scr.tile([P, T], F16, name="a_bf")
                                nc.vector.tensor_scalar_mul(out=a_bf[:, :ncols],
                                                            in0=pexp[:, :ncols],
                                                            scalar1=rden)
                                for tk in range(tq + 1):
                                    tp = pA.tile([P, P], F16, name="psA")
                                    nc.tensor.transpose(
                                        out=tp[:],
                                        in_=a_bf[:, tk * P:(tk + 1) * P],
                                        identity=identB[:])
                                    if (tq + tk) % 2 == 0:
                                        nc.vector.tensor_copy(
                                            out=at[:, tk, tq * P:(tq + 1) * P],
                                            in_=tp[:])
                                    else:
                                        nc.scalar.copy(
                                            out=at[:, tk, tq * P:(tq + 1) * P],
                                            in_=tp[:])
                            at_pair.append(at)
                        # O^T both heads (col-packed: out partitions 0-63/64-127)
                        o_ps = pA.tile([P, T], F32, name="psA")
                        for sub in range(2):
                            off = sub * DK
                            h = pair * 2 + sub
                            at = at_pair[sub]
                            nc.tensor.matmul(
                                o_ps[off:off + DK, :],
                                vvs[0][:, h * DK:(h + 1) * DK],
                                at[:, 0],
                                start=True, stop=False)
                            nc.tensor.matmul(
                                o_ps[off:off + DK, P:T],
                                vvs[1][:, h * DK:(h + 1) * DK],
                                at[:, 1, P:T],
                                start=False, stop=True)
                        if pair % 2 == 0:
                            nc.vector.tensor_copy(out=otp[pair][:], in_=o_ps[:])
                        else:
                            nc.scalar.copy(out=otp[pair][:], in_=o_ps[:])
                        # x0 += O_pair @ Wo_pair right away: x0 completes as
                        # soon as the last pair drains, so LN2(t0) overlaps
                        # the tq1 matmuls below.
                        nc.tensor.matmul(xps[0][:], otp[pair][:, 0:P],
                                         wo_t[:, pair],
                                         start=False, stop=(pair == KB - 1),
                                         skip_group_check=True)

                    # ---- x1 += O @ Wo (bridges LN2-t0 chain) ----
                    layernorm(0)
                    for kb in range(KB):
                        nc.tensor.matmul(xps[1][:],
                                         otp[kb][:, P:T],
                                         wo_t[:, kb],
                                         start=False, stop=(kb == KB - 1),
                                         skip_group_check=True)
                    transpose_tile(0)
                    layernorm(1)
                    # Pre-issue W1 t0-halves for fc<3 (pB ring): they need
                    # only the t0 transposes and bridge the LN2-t1 chain.
                    psf = []
                    for fc in range(3):
                        ps = pB.tile([P, T], F32, name="psB")
                        psf.append(ps)
                        for kb in range(KB):
                            nc.tensor.matmul(ps[:, 0:P],
                                             w1_t[:, kb, fc * P:(fc + 1) * P],
                                             xnT[kb][:, 0:P],
                                             start=(kb == 0), stop=False,
                                             skip_group_check=True)
                    transpose_tile(1)

                    def ffn_relu(fc, ps):
                        bias_ap = sw[:, 2 * KB + fc:2 * KB + fc + 1]
                        if fc % 2 == 0:
                            nc.scalar.activation(out=htf[fc][:], in_=ps[:],
                                                 func=AF.Relu, bias=bias_ap,
                                                 scale=1.0)
                        else:
                            nc.vector.tensor_scalar(out=htf[fc][:], in0=ps[:],
                                                    scalar1=bias_ap, scalar2=0.0,
                                                    op0=mybir.AluOpType.add,
                                                    op1=mybir.AluOpType.max)

                    # pass A: finish W1, relu, accumulate W2 into x0 only
                    for fc in range(FB):
                        if fc < 3:
                            ps = psf[fc]
                            for kb in range(KB):
                                nc.tensor.matmul(
                                    ps[:, P:T],
                                    w1_t[:, kb, fc * P:(fc + 1) * P],
                                    xnT[kb][:, P:T],
                                    start=(kb == 0), stop=(kb == KB - 1),
                                    skip_group_check=True)
                        else:
                            ps = pA.tile([P, T], F32, name="psA")
                            for kb in range(KB):
                                nc.tensor.matmul(
                                    ps[:],
                                    w1_t[:, kb, fc * P:(fc + 1) * P],
                                    xnT[kb][:],
                                    start=(kb == 0), stop=(kb == KB - 1))
                        ffn_relu(fc, ps)
                        nc.tensor.matmul(xps[0][:], htf[fc][:, 0:P],
                                         w2_t[:, fc], start=False, stop=False,
                                         skip_group_check=True)
                    nc.tensor.matmul(xps[0][:], ones1[:], rows_sb[:, 1],
                                     start=False, stop=True,
                                     skip_group_check=True)
                    # pass B: x1 accumulation (overlaps next LN's t0 chain)
                    for fc in range(FB):
                        nc.tensor.matmul(xps[1][:], htf[fc][:, P:T],
                                         w2_t[:, fc], start=False, stop=False,
                                         skip_group_check=True)
                    nc.tensor.matmul(xps[1][:], ones1[:], rows_sb[:, 1],
                                     start=False, stop=True,
                                     skip_group_check=True)

            # ================= final LN + chunked all-gather =================
            # Residual leaves PSUM first: the vocab pool reuses the xp banks,
            # and a PE write there while a final-LN read is in flight
            # (PE-W + DVE-R same bank) is fatal on HW. Reading from SBUF
            # chains every PSUM reader ahead of the vocab matmuls.
            ag_in = [dram.tile([D, P], F16, name=f"ag_in{c}") for c in range(2)]
            ag_out = [dram.tile([NCORES * D, P], F16, addr_space="Shared",
                                name=f"ag_out{c}") for c in range(2)]
            for t in range(TT):
                if t == 0:
                    nc.scalar.copy(out=xsb[t][:], in_=xps[t][:])
                else:
                    nc.vector.tensor_copy(out=xsb[t][:], in_=xps[t][:])
                layernorm(t, src=xsb)
                transpose_tile(t)
                for kb in range(KB):
                    nc.sync.dma_start(out=ag_in[t][kb * P:(kb + 1) * P, :],
                                      in_=xnT[kb][:, t * P:(t + 1) * P])
                nc.gpsimd.collective_compute(
                    "AllGather", mybir.AluOpType.bypass,
                    replica_groups=[list(range(NCORES))],
                    ins=[ag_in[t][:]], outs=[ag_out[t][:]])
            lps.close()   # free layer-phase PSUM for the vocab pool

            # ================= vocab projection (transposed) =================
            with tc.tile_pool(name="vw", bufs=6) as vw, \
                 tc.tile_pool(name="vo", bufs=8) as vo, \
                 tc.tile_pool(name="vg", bufs=1) as vg, \
                 tc.tile_pool(name="vps", bufs=2, space="PSUM") as vps:
                xgs = [vg.tile([P, KB, 2 * 512], F16, name=f"xg{c}")
                       for c in range(2)]
                for c in range(2):
                    src = ag_out[c][:].rearrange(
                        "(b kb p) t -> p kb b t", b=B, kb=KB, p=P)
                    for kb in range(KB):
                        nc.sync.dma_start(out=xgs[c][:, kb], in_=src[:, kb])

                def vocab_chunk(vc, ts_list):
                    wsb = vw.tile([P, KB, P], F16, name="wsb")
                    nc.sync.dma_start(
                        out=wsb,
                        in_=wout_h.ap()[vc].rearrange("p (kb n) -> p kb n", kb=KB))
                    ps = vps.tile([P, TS, 512], F32, name="vps")
                    for kb in range(KB):
                        for ts in ts_list:
                            nc.tensor.matmul(
                                ps[:, ts], wsb[:, kb],
                                xgs[ts // 2][:, kb, (ts % 2) * 512:
                                             (ts % 2 + 1) * 512],
                                start=(kb == 0), stop=(kb == KB - 1))
                    for ts in ts_list:
                        lg = vo.tile([P, 512], F16, name="lg")
                        if ts % 2 == 0:
                            nc.scalar.activation(out=lg, in_=ps[:, ts],
                                                 func=AF.Identity,
                                                 bias=bout_sb[:, vc:vc + 1],
                                                 scale=1.0)
                        else:
                            nc.vector.tensor_scalar_add(out=lg, in0=ps[:, ts],
                                                        scalar1=bout_sb[:, vc:vc + 1])
                        nc.sync.dma_start(out=logits_h.ap()[vc, ts], in_=lg)

                if os.environ.get("KERNEL_BISECT") == "novocab":
                    vocab_chunk(0, [0, 1, 2, 3])
                else:
                    for vc in range(VEARLY):
                        vocab_chunk(vc, [0, 1])
                    for vc in range(VEARLY, VC):
                        vocab_chunk(vc, [0, 1, 2, 3])
                    for vc in range(VEARLY):
                        vocab_chunk(vc, [2, 3])

    nc.compile()
    return nc


def _prep_inputs(inputs):
    """Host-side shard/cast/layout with exact LN folding."""
    f32 = np.float32
    f16 = np.float16

    idx = np.asarray(inputs["idx"])
    emb = np.asarray(inputs["emb"], f32)

    pos = np.arange(T, dtype=np.float64)[:, None]
    div = np.exp(np.arange(0, D, 2, dtype=np.float64) * (-math.log(10000.0) / D))
    pe = np.zeros((T, D), f32)
    pe[:, 0::2] = np.sin(pos * div).astype(f32)
    pe[:, 1::2] = np.cos(pos * div).astype(f32)

    # mask tiles: [TT, P, T]; tile tq row p masks cols > tq*128+p
    maskadd = np.zeros((TT, P, T), f32)
    for tq in range(TT):
        for p in range(P):
            maskadd[tq, p, tq * P + p + 1:] = NEGMASK

    wq = np.asarray(inputs["Wq"], f32)  # [L, H, D, DK]
    wk = np.asarray(inputs["Wk"], f32)
    wv = np.asarray(inputs["Wv"], f32)
    ln1g = np.asarray(inputs["ln1_g"], f32)  # [L, D]
    ln1b = np.asarray(inputs["ln1_b"], f32)
    ln2g = np.asarray(inputs["ln2_g"], f32)
    ln2b = np.asarray(inputs["ln2_b"], f32)
    w1 = np.asarray(inputs["W1"], f32)       # [L, D, DFF]
    b1 = np.asarray(inputs["b1"], f32)       # [L, DFF]
    b2 = np.asarray(inputs["b2"], f32)
    lnfg = np.asarray(inputs["lnf_g"], f32)
    lnfb = np.asarray(inputs["lnf_b"], f32)

    # concat heads: [L, D, D], then fold ln1 gain into rows
    wq_c = wq.transpose(0, 2, 1, 3).reshape(L, D, D)
    wk_c = wk.transpose(0, 2, 1, 3).reshape(L, D, D)
    wv_c = wv.transpose(0, 2, 1, 3).reshape(L, D, D)
    g1 = ln1g[:, :, None]
    wqkv = np.stack([wq_c * g1, wk_c * g1, wv_c * g1], axis=1)  # [L,3,D,D]
    cq = np.einsum('ld,ldo->lo', ln1b, wq_c)   # [L, D]
    ck = np.einsum('ld,ldo->lo', ln1b, wk_c)
    cv = np.einsum('ld,ldo->lo', ln1b, wv_c)
    w1_f = w1 * ln2g[:, :, None]
    b1_f = b1 + np.einsum('ld,ldf->lf', ln2b, w1)

    wqkv_t = np.ascontiguousarray(
        wqkv.reshape(L, 3, KB, P, D).transpose(0, 1, 3, 2, 4)).astype(f16)
    wo_t = np.ascontiguousarray(
        np.asarray(inputs["Wo"], f32).reshape(L, KB, P, D)
        .transpose(0, 2, 1, 3)).astype(f16)
    w1_t = np.ascontiguousarray(
        w1_f.reshape(L, KB, P, DFF).transpose(0, 2, 1, 3)).astype(f16)
    w2_t = np.ascontiguousarray(
        np.asarray(inputs["W2"], f32).reshape(L, FB, P, D)
        .transpose(0, 2, 1, 3)).astype(f16)

    smallw = np.zeros((L, P, 2 * KB + FB), f32)
    smallw[:, :, 0:KB] = cq.reshape(L, KB, P).transpose(0, 2, 1)
    smallw[:, :, KB:2 * KB] = ck.reshape(L, KB, P).transpose(0, 2, 1)
    smallw[:, :, 2 * KB:] = b1_f.reshape(L, FB, P).transpose(0, 2, 1)

    rows = np.stack([cv, b2], axis=1).astype(f16)  # [L, 2, D]

    # vocab: fold final LN gain/bias, pad each core's shard to 6400
    wout = np.asarray(inputs["Wout"], f32)
    bout = np.asarray(inputs["bout"], f32)
    wout_f = lnfg[:, None] * wout
    bout_f = bout + lnfb @ wout
    VPAD = VC * P * NCORES
    wout_pad = np.zeros((D, VPAD), f32)
    bout_pad = np.zeros((VPAD,), f32)
    for c in range(NCORES):
        lo, hi = c * VS, min((c + 1) * VS, V)
        if lo < V:
            wout_pad[:, c * VC * P:c * VC * P + (hi - lo)] = wout_f[:, lo:hi]
            bout_pad[c * VC * P:c * VC * P + (hi - lo)] = bout_f[lo:hi]

    common = dict(
        emb=emb.astype(f16), posenc=pe, maskadd=maskadd.astype(f16),
        wqkv=wqkv_t, wo=wo_t, w1=w1_t, w2=w2_t,
        smallw=smallw, rows=rows,
    )
    in_maps = []
    for c in range(NCORES):
        m = dict(common)
        m["idx"] = np.ascontiguousarray(idx[c].astype(np.int32).reshape(T, 1))
        ws = wout_pad[:, c * VC * P:(c + 1) * VC * P]  # [D, 6400]
        # wout_h[vc, p, kb*128+n] = ws[kb*128+p, vc*128+n]
        w4 = ws.reshape(KB, P, VC, P).transpose(2, 1, 0, 3).reshape(VC, P, KB * P)
        m["wout"] = np.ascontiguousarray(w4).astype(f16)
        bs = bout_pad[c * VC * P:(c + 1) * VC * P]
        m["bout"] = np.ascontiguousarray(bs.reshape(VC, P).T)
        in_maps.append(m)
    return in_maps


def _unshard(results):
    full = np.zeros((B, T, NCORES * VC * P), np.float32)
    for c in range(NCORES):
        arr = np.asarray(results[c]["logits"], np.float32)  # [VC, TS, P, 512]
        # ts = 2*chunk + b//4; col within ts-block = (b%4)*128 + t_local
        a = arr.reshape(VC, 2, 2, P, 4, P)      # (vc, c2, h2, p, b4, tl)
        a = a.transpose(2, 4, 1, 5, 0, 3)       # (h2, b4, c2, tl, vc, p)
        full[:, :, c * VC * P:(c + 1) * VC * P] = a.reshape(B, T, VC * P)
    out = np.zeros((B, T, V), np.float32)
    for c in range(NCORES):
        lo, hi = c * VS, min((c + 1) * VS, V)
        if lo < V:
            out[:, :, lo:hi] = full[:, :, c * VC * P:c * VC * P + (hi - lo)]
    return out


def kernel(**inputs):
    if "nc" not in _CACHE:
        _CACHE["nc"] = _build_program()
    nc = _CACHE["nc"]
    in_maps = _prep_inputs(inputs)

    if os.environ.get("KERNEL_USE_SIM"):
        from concourse.bass_interp import MultiCoreSim
        sim = MultiCoreSim(nc, num_cores=NCORES,
                           num_workers=int(os.environ.get("KERNEL_SIM_WORKERS", "8")))
        for c in range(NCORES):
            for name, val in in_maps[c].items():
                sim.cores[c].tensor(name)[:] = val
        sim.simulate()
        results = [
            {"logits": np.array(sim.cores[c].tensor("logits"))}
            for c in range(NCORES)
        ]
        return _unshard(results)

    res = bass_utils.run_bass_kernel_spmd(
        nc, in_maps, core_ids=list(range(NCORES)))
    return _unshard(res.results)
